# revision 17
# baseline (speedup 1.0000x reference)
"""Trainium2 Bass kernel: per-pixel 19x19 batch blur (KPN-style).

Reference computation:
    out[b,c,i,j] = (1/361) * sum_{ki,kj} pad[b,c,i+ki,j+kj] * kern[b, i*W+j, ki, kj]
with `pad` the 9-pixel reflection-padded input, shapes:
    input  (2, 3, 256, 256) f32
    kernel (2, 65536, 19, 19) f32    <- dominates memory traffic

Sharding: pure data parallel over (batch, H-tile): 8 cores = 2 batches x 4
tiles of 64 output rows each.

Hybrid DVE+ScalarE design (vs the f32 STT-only baseline at ~198us):
  The kernel is bound by per-pixel 361-tap dot products: 384 row-instrs/core
  (one per (row i, jblk, channel) over 128 pixel-partitions).  Measured facts
  on this silicon:
    - scalar_tensor_tensor (fused mul+accum) runs at 1x only: ~456ns/row.
    - plain tensor_tensor in fp16 hits the 2x_1P perf mode (~200ns/row when
      row-batched), but has no accumulator.
    - ScalarE activation(Copy, accum_out) reduces a 362-elem row in ~490ns,
      CONCURRENTLY with the DVE.
  So per 24-row block we split: 9 rows stay fused-STT on DVE; 15 rows are
  premultiplied on DVE via 2x tensor_tensor (channel-broadcast, row-batched)
  and reduced on ScalarE.  Both engines run ~7.3us/block in parallel.

  Layout: kern rows padded 361->362 (zero slot); patches as sliding strips
  SHIFTC[j, 19*r + kj] = pad[r, j+kj] in TWO copies: S1 shifted one element
  right, so odd output rows read at even (4B-aligned) offsets 19i+1 and
  even rows at 19i from S0 -- keeping every tensor_tensor operand run
  4B-aligned for the 2x mode.  fp16 operands, fp32 accumulation.
"""

import os
import sys

import numpy as np

for _p in ("/opt/trn_rl_repo", os.path.expanduser("~/.axon_site/_ro/trn_rl_repo")):
    if os.path.isdir(_p) and _p not in sys.path:
        sys.path.insert(0, _p)

from contextlib import ExitStack

from concourse import bacc, bass_utils, mybir, tile
from concourse.ap import AP

# Problem constants (hardcoded per the self-containment contract).
B, C, H, W = 2, 3, 256, 256
L = 19
PAD = L // 2  # 9
K2 = L * L  # 361
KP = K2 + 1  # 362: padded row length (slot 361 zeroed)
N_CORES = 8
ROWS_PER_CORE = H // 4  # 64  (4 H-tiles x 2 batches = 8 cores)
PR = ROWS_PER_CORE + 2 * PAD  # 82 padded rows per core
SF1 = L * PR + 2  # 1560 free elems per SHIFTC channel (2 pad)
R_CHUNK = 8  # output rows per kernel-DMA chunk
N_IBLK = ROWS_PER_CORE // R_CHUNK  # 8
KFREE = R_CHUNK * KP  # 2896
F32 = mybir.dt.float32
F16 = mybir.dt.float16

# Per-(block, channel) row split: B-rows premultiplied on DVE (2x TT) and
# reduced on ScalarE; A-rows fused STT on DVE.  Measured: STT ~456ns/row,
# TT-mult ~200ns/row, ACT-accum ~673ns/row (serialized accumulator chain),
# so a 12/12 split balances both engines at ~8us per 24-row block.
B_EVEN = (0, 2, 4, 6)  # read S0 at 19*i   (even offsets)
B_ODD = ()  # (odd B rows would read S1 at 19*i+1)
A_ROWS = (1, 3, 5, 7)  # fused STT, S1 (alignment irrelevant at 1x)

_CACHE: dict = {}


def _build_program():
    nc = bacc.Bacc(
        "TRN2",
        target_bir_lowering=False,
        debug=False,
        enable_asserts=False,
        num_devices=N_CORES,
    )
    kern = nc.dram_tensor("kern", [2 * N_IBLK * 128, KFREE], F16, kind="ExternalInput")
    # [copy(S0/S1), jb, j, c, f]
    shiftd = nc.dram_tensor("shiftc", [2, 2, 128, C * SF1], F16, kind="ExternalInput")
    # Separate outputs per accumulating engine (sharing one tile between
    # ScalarE and DVE accum writes costs ~90ns/ACT in ordering semaphores).
    NA = len(A_ROWS) * N_IBLK  # 24 A-cols per channel
    NB = (len(B_EVEN) + len(B_ODD)) * N_IBLK  # 40 B-cols per channel
    outd_v = nc.dram_tensor("out_v", [2 * 128, C * NA], F32, kind="ExternalOutput")
    outd_s = nc.dram_tensor("out_s", [2 * 128, C * NB], F32, kind="ExternalOutput")

    mult = mybir.AluOpType.mult
    copyfn = mybir.ActivationFunctionType.Copy
    NBE = len(B_EVEN)
    NBO = len(B_ODD)

    with tile.TileContext(nc) as tc, ExitStack() as ctx:
        cpool = ctx.enter_context(tc.tile_pool(name="cpool", bufs=1))
        kpool = ctx.enter_context(tc.tile_pool(name="kpool", bufs=3))
        spool = ctx.enter_context(tc.tile_pool(name="spool", bufs=2))
        qpool = ctx.enter_context(tc.tile_pool(name="qpool", bufs=3))
        # ScalarE is closer to PSUM: ACT garbage-out and accum land there.
        ppool = ctx.enter_context(tc.tile_pool(name="ppool", bufs=2, space="PSUM"))

        shiftc = {}

        def _load_shiftc(cp, jb):
            sc = cpool.tile([128, C * SF1], F16, name=f"shiftc_{cp}_{jb}")
            shiftc[(cp, jb)] = sc
            nc.sync.dma_start(
                out=sc[:, :],
                in_=AP(
                    shiftd,
                    (cp * 2 + jb) * 128 * C * SF1,
                    [(C * SF1, 128), (1, C * SF1)],
                ),
            )

        def _load_chunk(iblk, jb):
            kt = kpool.tile([128, KFREE], F16, name="kt", tag="kt")
            base = (iblk * 2 + jb) * 128 * KFREE
            nc.sync.dma_start(
                out=kt[:, :], in_=AP(kern, base, [(KFREE, 128), (1, KFREE)])
            )
            return kt

        # Critical-path first: block (0,0) inputs, then the rest.
        kts = {(0, 0): _load_chunk(0, 0)}
        _load_shiftc(0, 0)
        _load_shiftc(1, 0)
        _load_shiftc(0, 1)
        _load_shiftc(1, 1)

        outt_v = [cpool.tile([128, C * NA], F32, name=f"outtv{jb}") for jb in range(2)]
        outt_s = [cpool.tile([128, C * NB], F32, name=f"outts{jb}") for jb in range(2)]
        NBR = len(B_EVEN) + len(B_ODD)  # B-rows per (block, channel)

        for jb in range(2):
            for iblk in range(N_IBLK):
                kt = kts.pop((iblk, jb), None)
                if kt is None:
                    kt = _load_chunk(iblk, jb)
                s0 = shiftc[(0, jb)]
                s1 = shiftc[(1, jb)]

                # --- B rows: one 2x TT per parity group, channel-broadcast ---
                qe = qpool.tile([128, C * NBE * KP], F16, name="qe", tag="qe")
                i0 = iblk * R_CHUNK + B_EVEN[0]
                nc.vector.tensor_tensor(
                    out=AP(qe.tensor, 0, [(C * NBE * KP, 128), (NBE * KP, C), (KP, NBE), (1, KP)]),
                    in0=AP(kt.tensor, B_EVEN[0] * KP, [(KFREE, 128), (0, C), (2 * KP, NBE), (1, KP)]),
                    in1=AP(s0.tensor, L * i0, [(C * SF1, 128), (SF1, C), (2 * L, NBE), (1, KP)]),
                    op=mult,
                )
                if NBO:
                    qo = qpool.tile([128, C * NBO * KP], F16, name="qo", tag="qo")
                    i1 = iblk * R_CHUNK + B_ODD[0]
                    nc.vector.tensor_tensor(
                        out=AP(qo.tensor, 0, [(C * NBO * KP, 128), (NBO * KP, C), (KP, NBO), (1, KP)]),
                        in0=AP(kt.tensor, B_ODD[0] * KP, [(KFREE, 128), (0, C), (2 * KP, NBO), (1, KP)]),
                        in1=AP(s1.tensor, L * i1 + 1, [(C * SF1, 128), (SF1, C), (2 * L, NBO), (1, KP)]),
                        op=mult,
                    )

                # --- ScalarE reductions of the B rows ---
                for c in range(C):
                    for bi, il in enumerate(B_EVEN):
                        col = c * NB + iblk * NBR + bi
                        scr = ppool.tile([128, KP], F32, name="ascr", tag="ascr")
                        nc.scalar.activation(
                            out=scr[:, :],
                            in_=AP(qe.tensor, (c * NBE + bi) * KP, [(C * NBE * KP, 128), (1, KP)]),
                            func=copyfn,
                            scale=1.0 / K2,
                            accum_out=outt_s[jb][:, col : col + 1],
                        )
                    for bi, il in enumerate(B_ODD):
                        col = c * NB + iblk * NBR + NBE + bi
                        scr = ppool.tile([128, KP], F32, name="ascr", tag="ascr")
                        nc.scalar.activation(
                            out=scr[:, :],
                            in_=AP(qo.tensor, (c * NBO + bi) * KP, [(C * NBO * KP, 128), (1, KP)]),
                            func=copyfn,
                            scale=1.0 / K2,
                            accum_out=outt_s[jb][:, col : col + 1],
                        )

                # --- A rows: fused STT on DVE (1x) ---
                for c in range(C):
                    for ai, il in enumerate(A_ROWS):
                        i = iblk * R_CHUNK + il
                        col = c * NA + iblk * len(A_ROWS) + ai
                        scr = spool.tile([128, KP], F16, name="vscr", tag="vscr")
                        nc.vector.scalar_tensor_tensor(
                            out=scr[:, :],
                            in0=AP(kt.tensor, il * KP, [(KFREE, 128), (1, KP)]),
                            scalar=1.0 / K2,
                            in1=AP(s1.tensor, c * SF1 + L * i + 1, [(C * SF1, 128), (1, KP)]),
                            op0=mult,
                            op1=mult,
                            accum_out=outt_v[jb][:, col : col + 1],
                        )

            # jb's outputs stream out while the next jb computes.
            nc.sync.dma_start(
                out=AP(outd_v, jb * 128 * (C * NA), [(C * NA, 128), (1, C * NA)]),
                in_=outt_v[jb][:, :],
            )
            nc.sync.dma_start(
                out=AP(outd_s, jb * 128 * (C * NB), [(C * NB, 128), (1, C * NB)]),
                in_=outt_s[jb][:, :],
            )

    nc.compile()
    return nc


def _program():
    if "nc" not in _CACHE:
        _CACHE["nc"] = _build_program()
    return _CACHE["nc"]


def _shard_inputs(input, kernel):
    inp = np.ascontiguousarray(np.asarray(input, dtype=np.float32))
    kern16 = np.asarray(kernel, dtype=np.float32).astype(np.float16)

    # kern -> per-core chunks [b, q, (iblk, jb), j, il, k(362 zero-padded)]
    kr = kern16.reshape(B, 4, N_IBLK, R_CHUNK, 2, 128, K2)  # b q iblk il jb j k
    kr = np.pad(kr, ((0, 0),) * 6 + ((0, 1),))  # k -> 362, zero slot
    kr = kr.transpose(0, 1, 2, 4, 5, 3, 6)  # b q iblk jb j il k
    kr = np.ascontiguousarray(kr).reshape(B, 4, 2 * N_IBLK * 128, KFREE)

    pad = np.pad(inp, ((0, 0), (0, 0), (PAD, PAD), (PAD, PAD)), mode="reflect")
    pad16 = pad.astype(np.float16)
    # strips[b, c, r, j, kj] = pad[b, c, r, j + kj]
    strips = np.lib.stride_tricks.sliding_window_view(pad16, L, axis=3)
    in_maps = []
    for core in range(N_CORES):
        b, q = divmod(core, 4)
        r0 = q * ROWS_PER_CORE
        s = strips[b, :, r0 : r0 + PR, :, :]  # (C, PR, 256, L)
        s = s.transpose(2, 0, 1, 3).reshape(256, C, PR * L)  # (j2, c, 19r+kj)
        s0 = np.zeros((256, C, SF1), dtype=np.float16)
        s0[:, :, : PR * L] = s
        s1 = np.zeros_like(s0)
        s1[:, :, 1:] = s0[:, :, :-1]
        # dram layout [copy, jb, j, c*SF1]
        sc = np.stack([s0, s1]).reshape(2, 2, 128, C * SF1)
        in_maps.append({"kern": kr[b, q], "shiftc": np.ascontiguousarray(sc)})
    return in_maps


def _unshard_output(results):
    NAr, NBr = len(A_ROWS), len(B_EVEN) + len(B_ODD)
    b_order = list(B_EVEN) + list(B_ODD)
    out = np.empty((B, C, H, W), dtype=np.float32)
    for core in range(N_CORES):
        b, q = divmod(core, 4)
        # out_v: [jb*128+j, c*(8*3) + iblk*3 + ai], rows A_ROWS
        av = np.asarray(results[core]["out_v"]).reshape(2, 128, C, N_IBLK, NAr)
        asv = np.asarray(results[core]["out_s"]).reshape(2, 128, C, N_IBLK, NBr)
        rows = np.empty((2, 128, C, N_IBLK, R_CHUNK), dtype=np.float32)
        for ai, il in enumerate(A_ROWS):
            rows[..., il] = av[..., ai]
        for bi, il in enumerate(b_order):
            rows[..., il] = asv[..., bi]
        blk = rows.reshape(2, 128, C, ROWS_PER_CORE).transpose(2, 3, 0, 1)
        out[b, :, q * ROWS_PER_CORE : (q + 1) * ROWS_PER_CORE, :] = blk.reshape(
            C, ROWS_PER_CORE, W
        )
    return out


def run_sharded(inputs, **kw):
    """Run the compiled SPMD program; returns BassKernelResults (for profiling)."""
    in_maps = _shard_inputs(inputs["input"], inputs["kernel"])
    return bass_utils.run_bass_kernel_spmd(
        _program(), in_maps, core_ids=list(range(N_CORES)), **kw
    )


def kernel(input, kernel):
    res = run_sharded({"input": input, "kernel": kernel})
    return _unshard_output(res.results)
